# revision 26
# baseline (speedup 1.0000x reference)
"""Trainium2 Bass kernel for a Direct-Form-II-transposed IIR single-step update.

reference semantics (all fp32):
    out  = input * b[0] + v[..., 0]                  # [B, C]
    v_new[..., m] = input * b[m+1] - out * a[m]      # m = 0..7
    v_new[..., m] += v[..., m+1]   for m < 7
returns (out [B, C], v_new [B, C, 8])

Sharding: data-parallel over dim 0 (batch) across 8 NeuronCores.
Per core N = 32*32768 = 1,048,576 rows. Rows are processed in tiles of
[128 partitions x F_t rows]; tile sizes taper at the head/tail so the
first compute starts (and the last store finishes) on a small chunk.

Per tile (engines balanced so DMA ~ DVE ~ 200us/core):
  ACT:  tm_m = b[m+1] * x                  (8 scale-copies, per-partition scalar)
  DVE:  out  = b[0]*x + v[:,0]             (scalar_tensor_tensor)
        vn[:, m] = -a[m]*out + tm_m        (8 STT, strided column writes)
        vn[:, 0:7] += v[:, 1:8]            (one inner-unit-7 add; full DVE rate)
  DMA:  loads on the SP HWDGE ring, stores on the ACT HWDGE ring.
"""

from contextlib import ExitStack

import numpy as np

import concourse.bass as bass
import concourse.tile as tile
from concourse import bacc, bass2jax, mybir
from concourse.bass_utils import run_bass_kernel_spmd

NCORES = 8
B, C, M = 256, 32768, 8
BP = B // NCORES          # batch rows per core
N = BP * C                # rows per core
P = 128                   # SBUF partitions

# rows-per-partition per tile; sum must be N // P = 8192
F_LIST = [1024, 1024, 1024, 1024, 1024, 1024, 1024, 1024]
assert sum(F_LIST) == N // P

F32 = mybir.dt.float32

_cached = None


def _build():
    nc = bacc.Bacc(
        "TRN2",
        target_bir_lowering=False,
        debug=False,
        enable_asserts=False,
    )

    x_h = nc.dram_tensor("x", [1, N], F32, kind="ExternalInput")
    v_h = nc.dram_tensor("v", [1, N * M], F32, kind="ExternalInput")
    b_d = nc.dram_tensor("b", [1, M + 1], F32, kind="ExternalInput").ap()
    a_d = nc.dram_tensor("a", [1, M], F32, kind="ExternalInput").ap()
    o_h = nc.dram_tensor("o", [1, N], F32, kind="ExternalOutput")
    vn_h = nc.dram_tensor("vn", [1, N * M], F32, kind="ExternalOutput")

    mult = mybir.AluOpType.mult
    add = mybir.AluOpType.add
    Copy = mybir.ActivationFunctionType.Copy

    def row_ap(handle, off_rows, fcount, width):
        # [128, width*fcount] AP: partition p covers `fcount` rows of
        # `width` elems starting at flat row off_rows + p*fcount
        return bass.AP(
            handle,
            off_rows * width,
            [[fcount * width, P], [1, fcount * width]],
        )

    with tile.TileContext(nc) as tc, ExitStack() as ctx:
        cpool = ctx.enter_context(tc.tile_pool(name="coef", bufs=1))
        xpool = ctx.enter_context(tc.tile_pool(name="xin", bufs=3))
        vpool = ctx.enter_context(tc.tile_pool(name="vin", bufs=3))
        vnpool = ctx.enter_context(tc.tile_pool(name="vout", bufs=2))
        opool = ctx.enter_context(tc.tile_pool(name="oout", bufs=2))
        tmpool = ctx.enter_context(tc.tile_pool(name="ttmp", bufs=2))

        # --- coefficient prep (one-time) -------------------------------
        row = cpool.tile([1, 2 * M + 1], F32)
        nc.sync.dma_start(row[:, 0 : M + 1], b_d[:])
        nc.sync.dma_start(row[:, M + 1 : 2 * M + 1], a_d[:])
        rep = cpool.tile([P, 2 * M + 1], F32)
        nc.gpsimd.partition_broadcast(rep[:], row[:])
        na = cpool.tile([P, M], F32)  # -a, replicated per partition
        nc.vector.tensor_scalar_mul(na[:], rep[:, M + 1 : 2 * M + 1], -1.0)
        b0r = rep[:, 0:1]

        # --- main loop -------------------------------------------------
        off = 0
        for ti, F in enumerate(F_LIST):
            xtile = xpool.tile([P, F], F32, tag="xt")
            xt = xtile[:]
            vt = vpool.tile([P, F * M], F32, tag="vt")
            first = ti == 0
            last = ti == len(F_LIST) - 1
            H = F // 2

            def half_ap(handle, h, width):
                # half-h of this tile: partition stride stays F rows
                return bass.AP(
                    handle,
                    (off * P + h * H) * width,
                    [[F * width, P], [1, H * width]],
                )

            if first:
                # split tile-0 loads so compute starts after ~2 MB, not 4 MB
                nc.sync.dma_start(xt[:, 0:H], half_ap(x_h, 0, 1))
                nc.sync.dma_start(vt[:, 0 : H * M], half_ap(v_h, 0, M))
                nc.sync.dma_start(xt[:, H:F], half_ap(x_h, 1, 1))
                nc.sync.dma_start(vt[:, H * M :], half_ap(v_h, 1, M))
            else:
                nc.sync.dma_start(xt, row_ap(x_h, off * P, F, 1))
                nc.sync.dma_start(vt[:], row_ap(v_h, off * P, F, M))

            v3 = vt[:].rearrange("p (f m) -> p f m", m=M)
            vnt = vnpool.tile([P, F * M], F32, tag="vnt")
            vn3 = vnt[:].rearrange("p (f m) -> p f m", m=M)
            ot = opool.tile([P, F], F32, tag="ot")

            halves = [(0, H), (H, F)] if first else [(0, F)]
            for lo, hi in halves:
                s = slice(lo, hi)
                # out = x*b0 + v0
                nc.vector.scalar_tensor_tensor(
                    ot[:, s], xt[:, s], b0r, v3[:, s, 0], mult, add
                )
            for m in range(M):
                tm = tmpool.tile([P, F], F32, tag="tm")
                for lo, hi in halves:
                    s = slice(lo, hi)
                    nc.scalar.activation(
                        tm[:, s], xt[:, s], Copy, bias=0.0,
                        scale=rep[:, m + 1 : m + 2],
                    )
                    nc.vector.scalar_tensor_tensor(
                        vn3[:, s, m], ot[:, s], na[:, m : m + 1], tm[:, s], mult, add
                    )
            nc.scalar.dma_start(row_ap(o_h, off * P, F, 1), ot[:])
            # vn[:, :, 0:7] += v[:, :, 1:8] — inner-unit AP, full DVE rate.
            # Split shift+store on the last tile so the tail store is 2 MB.
            shift_halves = [(0, H), (H, F)] if last else [(0, F)]
            for h, (lo, hi) in enumerate(shift_halves):
                s = slice(lo, hi)
                nc.vector.tensor_add(
                    vn3[:, s, 0 : M - 1], vn3[:, s, 0 : M - 1], v3[:, s, 1:M]
                )
                if last:
                    nc.scalar.dma_start(
                        half_ap(vn_h, h, M), vnt[:, lo * M : hi * M]
                    )
            if not last:
                nc.scalar.dma_start(row_ap(vn_h, off * P, F, M), vnt[:])
            off += F

    nc.finalize()
    return nc


def _get_nc():
    global _cached
    if _cached is None:
        _cached = _build()
    return _cached


_runner = None


def _make_runner(nc):
    """Build the sharded executable ONCE (run_bass_via_pjrt re-traces and
    re-jits per call; this caches the jitted callable for repeat kernel()
    calls). Mirrors bass2jax.run_bass_via_pjrt's multi-core branch."""
    import jax
    from jax.experimental.shard_map import shard_map
    from jax.sharding import Mesh, PartitionSpec

    bass2jax.install_neuronx_cc_hook()

    partition_name = nc.partition_id_tensor.name if nc.partition_id_tensor else None
    in_names, out_names, out_avals, zero_outs = [], [], [], []
    for alloc in nc.m.functions[0].allocations:
        if not isinstance(alloc, mybir.MemoryLocationSet):
            continue
        name = alloc.memorylocations[0].name
        if alloc.kind == "ExternalInput":
            if name != partition_name:
                in_names.append(name)
        elif alloc.kind == "ExternalOutput":
            shape = tuple(alloc.tensor_shape)
            dtype = mybir.dt.np(alloc.dtype)
            out_names.append(name)
            out_avals.append(jax.core.ShapedArray(shape, dtype))
            zero_outs.append(np.zeros(shape, dtype))
    n_params = len(in_names)
    n_outs = len(out_avals)
    all_names = in_names + out_names + ([partition_name] if partition_name else [])
    donate = tuple(range(n_params, n_params + n_outs))

    def _body(*args):
        operands = list(args)
        if partition_name is not None:
            operands.append(bass2jax.partition_id_tensor())
        outs = bass2jax._bass_exec_p.bind(
            *operands,
            out_avals=tuple(out_avals),
            in_names=tuple(all_names),
            out_names=tuple(out_names),
            lowering_input_output_aliases=(),
            sim_require_finite=True,
            sim_require_nnan=True,
            nc=nc,
        )
        return tuple(outs)

    devices = jax.devices()[:NCORES]
    mesh = Mesh(np.asarray(devices), ("core",))
    in_specs = (PartitionSpec("core"),) * (n_params + n_outs)
    out_specs = (PartitionSpec("core"),) * n_outs
    sharded = jax.jit(
        shard_map(
            _body, mesh=mesh, in_specs=in_specs, out_specs=out_specs, check_rep=False
        ),
        donate_argnums=donate,
        keep_unused=True,
    )

    def run(in_maps):
        concat_in = [
            np.concatenate([np.asarray(m[n]) for m in in_maps], axis=0)
            for n in in_names
        ]
        concat_zeros = [
            np.zeros((NCORES * z.shape[0], *z.shape[1:]), z.dtype) for z in zero_outs
        ]
        out_arrs = sharded(*concat_in, *concat_zeros)
        return [
            {
                name: np.asarray(out_arrs[i]).reshape(NCORES, *out_avals[i].shape)[c]
                for i, name in enumerate(out_names)
            }
            for c in range(NCORES)
        ]

    return run


def _get_runner():
    global _runner
    if _runner is None:
        _runner = _make_runner(_get_nc())
    return _runner


def _run(input, v, b, a, trace=False, **spmd_kwargs):
    x = np.ascontiguousarray(np.asarray(input, dtype=np.float32)).reshape(B, C)
    vv = np.ascontiguousarray(np.asarray(v, dtype=np.float32))
    bb = np.ascontiguousarray(np.asarray(b, dtype=np.float32)).reshape(1, M + 1)
    aa = np.ascontiguousarray(np.asarray(a, dtype=np.float32)).reshape(1, M)

    in_maps = []
    for c in range(NCORES):
        xs = x[c * BP : (c + 1) * BP].reshape(1, N)
        vs = vv[c * BP : (c + 1) * BP].reshape(1, N * M)
        in_maps.append({"x": xs, "v": vs, "b": bb, "a": aa})

    if trace or spmd_kwargs:
        res = run_bass_kernel_spmd(
            _get_nc(), in_maps, list(range(NCORES)), trace=trace, **spmd_kwargs
        )
        results = res.results
    else:
        results = _get_runner()(in_maps)
        res = None

    out = np.empty((B, C), dtype=np.float32)
    v_new = np.empty((B, C, M), dtype=np.float32)
    for c in range(NCORES):
        out[c * BP : (c + 1) * BP] = results[c]["o"].reshape(BP, C)
        v_new[c * BP : (c + 1) * BP] = results[c]["vn"].reshape(BP, C, M)
    return (out, v_new), res


def kernel(input, v, b, a):
    (out, v_new), _ = _run(input, v, b, a)
    return out, v_new


# revision 27
# speedup vs baseline: 1.0753x; 1.0753x over previous
"""Trainium2 Bass kernel for a Direct-Form-II-transposed IIR single-step update.

reference semantics (all fp32):
    out  = input * b[0] + v[..., 0]                  # [B, C]
    v_new[..., m] = input * b[m+1] - out * a[m]      # m = 0..7
    v_new[..., m] += v[..., m+1]   for m < 7
returns (out [B, C], v_new [B, C, 8])

Sharding: data-parallel over dim 0 (batch) across 8 NeuronCores.
Per core N = 32*32768 = 1,048,576 rows. Rows are processed in tiles of
[128 partitions x F_t rows]; tile sizes taper at the head/tail so the
first compute starts (and the last store finishes) on a small chunk.

Per tile (engines balanced so DMA ~ DVE ~ 200us/core):
  ACT:  tm_m = b[m+1] * x                  (8 scale-copies, per-partition scalar)
  DVE:  out  = b[0]*x + v[:,0]             (scalar_tensor_tensor)
        vn[:, m] = -a[m]*out + tm_m        (8 STT, strided column writes)
        vn[:, 0:7] += v[:, 1:8]            (one inner-unit-7 add; full DVE rate)
  DMA:  loads on the SP HWDGE ring, stores on the ACT HWDGE ring.
"""

from contextlib import ExitStack

import numpy as np

import concourse.bass as bass
import concourse.tile as tile
from concourse import bacc, bass2jax, mybir
from concourse.bass_utils import run_bass_kernel_spmd

NCORES = 8
B, C, M = 256, 32768, 8
BP = B // NCORES          # batch rows per core
N = BP * C                # rows per core
P = 128                   # SBUF partitions

# rows-per-partition per tile; sum must be N // P = 8192
F_LIST = [1024, 1024, 1024, 1024, 1024, 1024, 1024, 1024]
assert sum(F_LIST) == N // P

F32 = mybir.dt.float32

_cached = None


def _build():
    nc = bacc.Bacc(
        "TRN2",
        target_bir_lowering=False,
        debug=False,
        enable_asserts=False,
    )

    x_h = nc.dram_tensor("x", [1, N], F32, kind="ExternalInput")
    v_h = nc.dram_tensor("v", [1, N * M], F32, kind="ExternalInput")
    b_d = nc.dram_tensor("b", [1, M + 1], F32, kind="ExternalInput").ap()
    a_d = nc.dram_tensor("a", [1, M], F32, kind="ExternalInput").ap()
    o_h = nc.dram_tensor("o", [1, N], F32, kind="ExternalOutput")
    vn_h = nc.dram_tensor("vn", [1, N * M], F32, kind="ExternalOutput")

    mult = mybir.AluOpType.mult
    add = mybir.AluOpType.add
    Copy = mybir.ActivationFunctionType.Copy

    def row_ap(handle, off_rows, fcount, width):
        # [128, width*fcount] AP: partition p covers `fcount` rows of
        # `width` elems starting at flat row off_rows + p*fcount
        return bass.AP(
            handle,
            off_rows * width,
            [[fcount * width, P], [1, fcount * width]],
        )

    with tile.TileContext(nc) as tc, ExitStack() as ctx:
        cpool = ctx.enter_context(tc.tile_pool(name="coef", bufs=1))
        xpool = ctx.enter_context(tc.tile_pool(name="xin", bufs=3))
        vpool = ctx.enter_context(tc.tile_pool(name="vin", bufs=3))
        vnpool = ctx.enter_context(tc.tile_pool(name="vout", bufs=2))
        opool = ctx.enter_context(tc.tile_pool(name="oout", bufs=2))
        tmpool = ctx.enter_context(tc.tile_pool(name="ttmp", bufs=2))

        # --- coefficient prep (one-time) -------------------------------
        row = cpool.tile([1, 2 * M + 1], F32)
        nc.sync.dma_start(row[:, 0 : M + 1], b_d[:])
        nc.sync.dma_start(row[:, M + 1 : 2 * M + 1], a_d[:])
        rep = cpool.tile([P, 2 * M + 1], F32)
        nc.gpsimd.partition_broadcast(rep[:], row[:])
        na = cpool.tile([P, M], F32)  # -a, replicated per partition
        nc.vector.tensor_scalar_mul(na[:], rep[:, M + 1 : 2 * M + 1], -1.0)
        b0r = rep[:, 0:1]

        # --- main loop -------------------------------------------------
        off = 0
        for ti, F in enumerate(F_LIST):
            xtile = xpool.tile([P, F], F32, tag="xt")
            xt = xtile[:]
            nc.sync.dma_start(xt, row_ap(x_h, off * P, F, 1))
            vt = vpool.tile([P, F * M], F32, tag="vt")
            nc.sync.dma_start(vt[:], row_ap(v_h, off * P, F, M))

            v3 = vt[:].rearrange("p (f m) -> p f m", m=M)
            vnt = vnpool.tile([P, F * M], F32, tag="vnt")
            vn3 = vnt[:].rearrange("p (f m) -> p f m", m=M)
            ot = opool.tile([P, F], F32, tag="ot")

            # out = x*b0 + v0
            nc.vector.scalar_tensor_tensor(ot[:], xt, b0r, v3[:, :, 0], mult, add)
            for m in range(M):
                # tm = b[m+1]*x on the (otherwise idle) scalar engine
                tm = tmpool.tile([P, F], F32, tag="tm")
                nc.scalar.activation(
                    tm[:], xt, Copy, bias=0.0, scale=rep[:, m + 1 : m + 2]
                )
                # vn[:, m] = -a[m]*out + tm
                nc.vector.scalar_tensor_tensor(
                    vn3[:, :, m], ot[:], na[:, m : m + 1], tm[:], mult, add
                )
            # vn[:, :, 0:7] += v[:, :, 1:8] — inner-unit AP, full DVE rate
            nc.vector.tensor_add(
                vn3[:, :, 0 : M - 1], vn3[:, :, 0 : M - 1], v3[:, :, 1:M]
            )

            nc.scalar.dma_start(row_ap(o_h, off * P, F, 1), ot[:])
            nc.scalar.dma_start(row_ap(vn_h, off * P, F, M), vnt[:])
            off += F

    nc.finalize()
    return nc


def _get_nc():
    global _cached
    if _cached is None:
        _cached = _build()
    return _cached


_runner = None


def _make_runner(nc):
    """Build the sharded executable ONCE (run_bass_via_pjrt re-traces and
    re-jits per call; this caches the jitted callable for repeat kernel()
    calls). Mirrors bass2jax.run_bass_via_pjrt's multi-core branch."""
    import jax
    from jax.experimental.shard_map import shard_map
    from jax.sharding import Mesh, PartitionSpec

    bass2jax.install_neuronx_cc_hook()

    partition_name = nc.partition_id_tensor.name if nc.partition_id_tensor else None
    in_names, out_names, out_avals, zero_outs = [], [], [], []
    for alloc in nc.m.functions[0].allocations:
        if not isinstance(alloc, mybir.MemoryLocationSet):
            continue
        name = alloc.memorylocations[0].name
        if alloc.kind == "ExternalInput":
            if name != partition_name:
                in_names.append(name)
        elif alloc.kind == "ExternalOutput":
            shape = tuple(alloc.tensor_shape)
            dtype = mybir.dt.np(alloc.dtype)
            out_names.append(name)
            out_avals.append(jax.core.ShapedArray(shape, dtype))
            zero_outs.append(np.zeros(shape, dtype))
    n_params = len(in_names)
    n_outs = len(out_avals)
    all_names = in_names + out_names + ([partition_name] if partition_name else [])
    donate = tuple(range(n_params, n_params + n_outs))

    def _body(*args):
        operands = list(args)
        if partition_name is not None:
            operands.append(bass2jax.partition_id_tensor())
        outs = bass2jax._bass_exec_p.bind(
            *operands,
            out_avals=tuple(out_avals),
            in_names=tuple(all_names),
            out_names=tuple(out_names),
            lowering_input_output_aliases=(),
            sim_require_finite=True,
            sim_require_nnan=True,
            nc=nc,
        )
        return tuple(outs)

    devices = jax.devices()[:NCORES]
    mesh = Mesh(np.asarray(devices), ("core",))
    in_specs = (PartitionSpec("core"),) * (n_params + n_outs)
    out_specs = (PartitionSpec("core"),) * n_outs
    sharded = jax.jit(
        shard_map(
            _body, mesh=mesh, in_specs=in_specs, out_specs=out_specs, check_rep=False
        ),
        donate_argnums=donate,
        keep_unused=True,
    )

    def run(in_maps):
        concat_in = [
            np.concatenate([np.asarray(m[n]) for m in in_maps], axis=0)
            for n in in_names
        ]
        concat_zeros = [
            np.zeros((NCORES * z.shape[0], *z.shape[1:]), z.dtype) for z in zero_outs
        ]
        out_arrs = sharded(*concat_in, *concat_zeros)
        return [
            {
                name: np.asarray(out_arrs[i]).reshape(NCORES, *out_avals[i].shape)[c]
                for i, name in enumerate(out_names)
            }
            for c in range(NCORES)
        ]

    return run


def _get_runner():
    global _runner
    if _runner is None:
        _runner = _make_runner(_get_nc())
    return _runner


def _run(input, v, b, a, trace=False, **spmd_kwargs):
    x = np.ascontiguousarray(np.asarray(input, dtype=np.float32)).reshape(B, C)
    vv = np.ascontiguousarray(np.asarray(v, dtype=np.float32))
    bb = np.ascontiguousarray(np.asarray(b, dtype=np.float32)).reshape(1, M + 1)
    aa = np.ascontiguousarray(np.asarray(a, dtype=np.float32)).reshape(1, M)

    in_maps = []
    for c in range(NCORES):
        xs = x[c * BP : (c + 1) * BP].reshape(1, N)
        vs = vv[c * BP : (c + 1) * BP].reshape(1, N * M)
        in_maps.append({"x": xs, "v": vs, "b": bb, "a": aa})

    if trace or spmd_kwargs:
        res = run_bass_kernel_spmd(
            _get_nc(), in_maps, list(range(NCORES)), trace=trace, **spmd_kwargs
        )
        results = res.results
    else:
        results = _get_runner()(in_maps)
        res = None

    out = np.empty((B, C), dtype=np.float32)
    v_new = np.empty((B, C, M), dtype=np.float32)
    for c in range(NCORES):
        out[c * BP : (c + 1) * BP] = results[c]["o"].reshape(BP, C)
        v_new[c * BP : (c + 1) * BP] = results[c]["vn"].reshape(BP, C, M)
    return (out, v_new), res


def kernel(input, v, b, a):
    (out, v_new), _ = _run(input, v, b, a)
    return out, v_new
